# revision 19
# baseline (speedup 1.0000x reference)
"""TRN2 Bass kernel for nn_MultiHeadAttention_86878598464357.

reference:  qkv = x @ w_qkv.T (RoPE on q,k) -> causal softmax attention ->
            torch-faithful reshape [B,H,T,D]->[B,T,C] -> proj @ w_proj.T

Sharding (8 NeuronCores): tensor-parallel over heads, 2 heads per core.
Because the torch-faithful reshape makes output row t' depend only on head
t'//128, each core independently computes full output rows for its heads --
no collectives.

Per core (all fp16 operands, fp32 PSUM accumulation):
  - qkv projection for its 2 heads
  - RoPE: one scalar PSUM->SBUF fp16 copy, then 4 fp16 vector ops against
    host-precomputed duplicated cos/sin tables (2x DVE mode)
  - causal attention in transposed-score layout S^T[s,t]: exp on scalar
    engine (scores O(6), fp32-safe without max subtraction), denominator
    via per-j ones-matmul accumulated in PSUM, reciprocal via [1,TB] ->
    [128,TB//128] DMA round-trip, gpsimd partition-broadcast, normalize TT
  - output projection with stride-16 lhsT access implementing the reshape
Emission order software-pipelines: attention starts inside the qkv(0)
phase, qkv(1) weaves with remaining attn(0), proj weaves with attn(1),
and the denominator chain of block i is emitted between j-units of block
i+1 so no engine queue blocks on the DMA round-trip latency.
"""
import math
from contextlib import ExitStack

import numpy as np

B, T, C = 2, 2048, 2048
H, D = 16, 128
HL = 2
TB = 512
NTB = T // TB
NTT = T // 128
KT = C // 128
SCALE = 1.0 / math.sqrt(D)
N_CORES = 8

_CACHE = {}


def _take(gen, n):
    """Pull and run up to n units from a generator of thunks."""
    for _ in range(n):
        f = next(gen, None)
        if f is None:
            return False
        f()
    return True


def _weave(gen_a, gen_b, na, nb):
    """Round-robin: na units from a, nb units from b, until both dry."""
    alive_a = alive_b = True
    while alive_a or alive_b:
        if alive_a:
            alive_a = _take(gen_a, na)
        if alive_b:
            alive_b = _take(gen_b, nb)


def _emit(nc, io, p, mybir):
    F32 = mybir.dt.float32
    F16 = mybir.dt.float16

    # ---- constants + ACT table prefetch ----
    ones_sb = p["const"].tile([128, 1], F16, name="ones_sb")
    nc.vector.memset(ones_sb[:], 1.0)
    ones_f32 = p["const"].tile([128, 1], F32, name="ones_f32")
    nc.vector.memset(ones_f32[:], 1.0)
    warm = p["const"].tile([128, 1], F32, name="warm")
    # dummy exp: forces the ACT table load at t=0, hidden under initial DMAs
    nc.scalar.activation(warm[:], ones_sb[:], mybir.ActivationFunctionType.Exp)

    w_sb = p["const"].tile([128, KT, 6 * 128], F16, name="w_sb")

    def load_w(fp, q, eng=None):
        ks = slice(q * 4, q * 4 + 4)
        (eng or nc.sync).dma_start(
            out=w_sb[:, ks, fp * 256 : (fp + 1) * 256],
            in_=io["w_qkv_t"][
                ks.start * 128 : ks.stop * 128, fp * 2 : fp * 2 + 2
            ].rearrange("(kt p) f d -> p kt (f d)", p=128),
        )

    cc_sb = p["const"].tile([128, T], F16, name="cc_sb")
    ss_sb = p["const"].tile([128, T], F16, name="ss_sb")
    tri_sb = p["const"].tile([128, 128], F16, name="tri_sb")

    def load_tables():
        nc.gpsimd.dma_start(out=cc_sb[:], in_=io["cc"][:])
        nc.gpsimd.dma_start(out=ss_sb[:], in_=io["ss"][:])
        nc.gpsimd.dma_start(out=tri_sb[:], in_=io["tri"][:])

    def wslice(kt, fb):
        return w_sb[:, kt, fb * 128 : (fb + 1) * 128]

    qkv_t = {}
    out_sb = {}
    last_dn = {}

    # ---------------- qkv projection + rope ----------------
    def qkv_units(b):
        q = {h: p["qkvp"].tile([128, T], F16, name=f"q_sb_{h}") for h in range(HL)}
        k = {h: p["qkvp"].tile([128, T], F16, name=f"k_sb_{h}") for h in range(HL)}
        v = p["qkvp"].tile([128, NTT, HL * 128], F16, name="v_sb")
        qkv_t[b] = (q, k, v)
        x_holder = {}

        def load_x(tb, quarter=None):
            def f():
                if quarter is None or quarter == 0:
                    x_holder[tb] = p["xp"].tile([128, KT, TB], F16, name="x_sb")
                x_sb = x_holder[tb]
                ks = (slice(0, KT) if quarter is None
                      else slice(quarter * 4, quarter * 4 + 4))
                nc.sync.dma_start(
                    out=x_sb[:, ks],
                    in_=io["x_t"][b][
                        ks.start * 128 : ks.stop * 128, tb * TB : (tb + 1) * TB
                    ].rearrange("(kt p) t -> p kt t", p=128),
                )
            return f

        psum_hold = {}

        def qk_mms(tb, fb, k0, k1):
            def f():
                x_sb = x_holder[tb]
                if k0 == 0:
                    psum_hold[fb] = p["ps_mm"].tile([128, TB], F32,
                                                    name="qk_psum", tag="mmps")
                psum = psum_hold[fb]
                for kt in range(k0, k1):
                    nc.tensor.matmul(
                        psum[:],
                        wslice(kt, fb),
                        rhs=x_sb[:, kt],
                        start=(kt == 0),
                        stop=(kt == KT - 1),
                    )
            return f

        def qk_rope(tb, fb, dst, h):
            def f():
                ts = slice(tb * TB, (tb + 1) * TB)
                psum = psum_hold.pop(fb)
                # rope: dst = psum*cc + swap(psum)*ss; swap via scalar copies
                # (PSUM operands are exempt from the equal-base-partition rule)
                qsw = p["rp"].tile([128, TB], F16, name="qsw")
                nc.scalar.copy(qsw[0:64, :], psum[64:128, :])
                nc.scalar.copy(qsw[64:128, :], psum[0:64, :])
                t1 = p["rp"].tile([128, TB], F16, name="rope_t1")
                t2 = p["rp"].tile([128, TB], F16, name="rope_t2")
                nc.vector.tensor_mul(t1[:], psum[:], cc_sb[:, ts])
                nc.vector.tensor_mul(t2[:], qsw[:], ss_sb[:, ts])
                nc.vector.tensor_add(dst[h][:, ts], t1[:], t2[:])
            return f

        def qk_chunk(tb, fb, dst, h):
            mm = qk_mms(tb, fb, 0, KT)
            rp = qk_rope(tb, fb, dst, h)
            def f():
                mm()
                rp()
            return f

        def v_chunk(tb, tl):
            def f():
                x_sb = x_holder[tb]
                tt = tb * 4 + tl
                psum = p["ps_mm"].tile([128, HL * 128], F32, name="v_psum",
                                       tag="mmps")
                for kt in range(KT):
                    nc.tensor.matmul(
                        psum[:],
                        x_sb[:, kt, tl * 128 : (tl + 1) * 128],
                        rhs=w_sb[:, kt, 4 * 128 : 6 * 128],
                        start=(kt == 0),
                        stop=(kt == KT - 1),
                    )
                nc.scalar.copy(v[:, tt], psum[:])
            return f

        fbs = [(q, 0), (q, 1), (k, 0), (k, 1)]
        for tb in range(NTB):
            if b == 0 and tb == 0:
                # cold start: minimal-data half-chains so PE starts after
                # only x[kt0-7] (1MB) + w[q-pair,kt0-7] (512KB) have landed
                yield load_x(tb, quarter=0)
                yield load_x(tb, quarter=1)
                yield lambda: load_w(0, 0)
                yield lambda: load_w(0, 1)
                yield load_x(tb, quarter=2)
                yield load_x(tb, quarter=3)
                yield lambda: load_w(0, 2, nc.gpsimd)
                yield lambda: load_w(0, 3, nc.gpsimd)
                yield lambda: load_w(1, 0)
                yield lambda: load_w(1, 1)
                yield lambda: load_w(1, 2)
                yield lambda: load_w(1, 3)
                yield load_tables
                for q_ in range(4):
                    yield lambda q_=q_: load_w(2, q_)
                for pair in (0, 2):
                    a, b_ = pair, pair + 1
                    yield qk_mms(tb, a, 0, 8)
                    yield qk_mms(tb, b_, 0, 8)
                    yield qk_mms(tb, a, 8, KT)
                    yield qk_rope(tb, a, fbs[a][0], fbs[a][1])
                    yield qk_mms(tb, b_, 8, KT)
                    yield qk_rope(tb, b_, fbs[b_][0], fbs[b_][1])
            else:
                yield load_x(tb)
                for fb, (dst, h) in enumerate(fbs):
                    yield qk_chunk(tb, fb, dst, h)
            for tl in range(4):
                yield v_chunk(tb, tl)

    # ---------------- attention ----------------
    def alloc_out(b):
        for h in range(HL):
            out_sb[(b, h)] = p["outp"].tile([128, T], F16, name=f"o_sb_{b}_{h}")

    def attn_units(b, blocks):
        """Yield j-units for the given (h, tb) blocks; denominator epilogues
        of block i are yielded interleaved between units of block i+1."""
        pending = []  # deferred epilogue thunks

        for h, tb in blocks:
            q, k, v = qkv_t[b]
            o_sb = out_sb[(b, h)]
            ts = slice(tb * TB, (tb + 1) * TB)
            o_ps = p["ps_o"].tile([128, TB], F32, name="o_ps", tag="ops")
            ea = {}
            njs = tb * 4 + 4

            for j in range(njs):
                def f(h=h, tb=tb, j=j, o_ps=o_ps, ea=ea, njs=njs):
                    c0 = max(0, j * 128 - tb * TB)
                    cs = slice(c0, TB)
                    tcs = slice(tb * TB + c0, (tb + 1) * TB)
                    s_ps = p["ps_s"].tile([128, TB], F32, name="s_ps", tag="sps")
                    nc.tensor.matmul(
                        s_ps[:, cs],
                        k[h][:, j * 128 : (j + 1) * 128],
                        rhs=q[h][:, tcs],
                        start=True,
                        stop=True,
                    )
                    e_sb = p["ep"].tile([128, TB], F16, name="e_sb", tag="e")
                    nc.scalar.activation(
                        e_sb[:, cs],
                        s_ps[:, cs],
                        mybir.ActivationFunctionType.Exp,
                        scale=SCALE,
                    )
                    if j >= tb * 4:
                        dcs = slice(c0, c0 + 128)
                        nc.vector.tensor_mul(e_sb[:, dcs], e_sb[:, dcs], tri_sb[:])
                    nc.tensor.matmul(
                        o_ps[:, cs],
                        v[:, j, h * 128 : (h + 1) * 128],
                        rhs=e_sb[:, cs],
                        start=(j == 0),
                        stop=(j == njs - 1),
                    )
                    # denominator partials accumulate on the vector engine
                    if j == 0:
                        ea["t"] = p["eap"].tile([128, TB], F16, name="eacc")
                        nc.vector.tensor_copy(ea["t"][:], e_sb[:])
                    else:
                        nc.vector.tensor_add(
                            ea["t"][:, cs], ea["t"][:, cs], e_sb[:, cs]
                        )
                yield f
                if pending and j % 2 == 1:
                    yield pending.pop(0)

            # epilogue for this block, deferred into the next block's units
            dn = {}

            def epi1(ea=ea, dn=dn):
                d_ps = p["ps_d"].tile([1, TB], F32, name="d_ps", tag="dps")
                nc.tensor.matmul(d_ps[:], ones_sb[:], rhs=ea["t"][:],
                                 start=True, stop=True)
                dn["den_sb"] = p["dnp"].tile([1, TB], F32, name="den_sb")
                nc.vector.tensor_copy(dn["den_sb"][:], d_ps[:])
                last_dn["dn"] = dn
                dn["den128"] = p["dnp"].tile([128, TB // 128], F32, name="den128")
                nc.sync.dma_start(out=dn["den128"][:], in_=dn["den_sb"][0:1, :])

            def epi2(dn=dn):
                dn["rec128"] = p["dnp"].tile([128, TB // 128], F32, name="rec128")
                nc.vector.reciprocal(dn["rec128"][:], dn["den128"][:])
                dn["rec_sb"] = p["dnp"].tile([1, TB], F32, name="rec_sb")
                nc.sync.dma_start(out=dn["rec_sb"][0:1, :], in_=dn["rec128"][:])

            def epi3(dn=dn, o_ps=o_ps, o_sb=o_sb, ts=ts):
                rb = p["dnp"].tile([128, TB], F32, name="rb_sb")
                nc.gpsimd.partition_broadcast(rb[:], dn["rec_sb"][:])
                nc.vector.tensor_mul(o_sb[:, ts], o_ps[:], rb[:])

            pending.extend([epi1, epi2, epi3])

        for f in pending:
            yield f

    # ---------------- output projection ----------------
    wt_tiles = {}
    wt_order = []

    def proj_units(jobs, gate_first=False, split_loads=False):
        """jobs: list of (b, ob, h); loads wt when ob not resident (bufs=3).
        gate_first: prefix the first chunk with a 1-col matmul reading the
        last attention block's denominator -- FIFO-forces these chunks into
        the kernel-tail window instead of being hoisted as gap fillers."""
        gate = [gate_first]

        def load_wt(ob):
            def f():
                wt = p["wp"].tile([128, KT, TB], F16, name="wt")
                if split_loads:
                    for half, eng in ((0, nc.sync), (1, nc.gpsimd)):
                        ks = slice(half * 8, half * 8 + 8)
                        eng.dma_start(
                            out=wt[:, ks],
                            in_=io["w_proj_t"][
                                ks.start * 128 : ks.stop * 128,
                                ob * TB : (ob + 1) * TB,
                            ].rearrange("(kt p) o -> p kt o", p=128),
                        )
                else:
                    nc.sync.dma_start(
                        out=wt[:],
                        in_=io["w_proj_t"][:, ob * TB : (ob + 1) * TB].rearrange(
                            "(kt p) o -> p kt o", p=128
                        ),
                    )
                wt_tiles[ob] = wt
            return f

        def pchunk(b, ob, h):
            def f():
                wt = wt_tiles[ob]
                os_ = slice(ob * TB, (ob + 1) * TB)
                y_ps = p["ps_mm"].tile([128, TB], F32, name="y_ps", tag="mmps")
                if gate[0]:
                    gate[0] = False
                    nc.tensor.matmul(
                        y_ps[0:1, 0:1], ones_f32[0:1, 0:1],
                        rhs=last_dn["dn"]["den_sb"][0:1, 0:1],
                        start=True, stop=True,
                    )
                o_sb = out_sb[(b, h)]
                for kt in range(KT):
                    lhsT = o_sb.rearrange("d (t2 g) -> d g t2", g=16)[:, kt]
                    nc.tensor.matmul(
                        y_ps[:],
                        lhsT,
                        rhs=wt[:, kt],
                        start=(kt == 0),
                        stop=(kt == KT - 1),
                    )
                y_sb = p["yp"].tile([128, TB], F32, name="y_sb")
                nc.vector.tensor_copy(y_sb[:], y_ps[:])
                nc.gpsimd.dma_start(out=io["y"][b, h, :, os_], in_=y_sb[:])
            return f

        for b, ob, h in jobs:
            if ob not in wt_order[-3:]:   # mirrors wp pool bufs=3 residency
                yield load_wt(ob)
                wt_order.append(ob)
            yield pchunk(b, ob, h)

    # ---------------- schedule ----------------
    NOB = C // TB
    g_qkv0 = qkv_units(0)
    _take(g_qkv0, 35)                       # tb=0 fully
    alloc_out(0)
    g_attn0 = attn_units(0, [(h, tb) for tb in range(NTB) for h in range(HL)])
    _weave(g_qkv0, g_attn0, 3, 3)           # qkv(0) tb1-3 with early attn(0)
    g_qkv1 = qkv_units(1)
    _weave(g_qkv1, g_attn0, 1, 2)           # qkv(1) with remaining attn(0)

    alloc_out(1)
    # attn(1): h0's smallest block goes LAST so that by the time it ends,
    # h1 is complete and ~20us of proj(1,h1)+proj(0,ob3) chunks sit behind
    # it in the PE FIFO, covering the trailing denominator chains.
    g_attn1 = attn_units(
        1, [(0, 3), (0, 2), (0, 1)] + [(1, tb) for tb in (3, 2, 1, 0)]
        + [(0, 0)]
    )
    g_proj0 = proj_units([(0, ob, h) for ob in range(3) for h in range(HL)])
    _weave(g_attn1, g_proj0, 11, 1)         # attn(1) with proj(0,ob0-2)

    # mid chunks: ready as soon as h1 closes; cover the last den chains
    # (wp ring: ld3 evicts ob0's slot right after its last consumer)
    _take(proj_units([(1, 0, 1), (0, 3, 0), (0, 3, 1), (1, 3, 1),
                      (1, 1, 1), (1, 2, 1)]), 100)
    # tail: proj(1,h0); single reload (ob0) evicts the just-consumed slot
    _take(proj_units([(1, 1, 0), (1, 2, 0), (1, 3, 0), (1, 0, 0)],
                     split_loads=True), 100)


def _build():
    from concourse import bacc
    import concourse.mybir as mybir
    import concourse.tile as tile

    F32 = mybir.dt.float32
    F16 = mybir.dt.float16

    nc = bacc.Bacc(None, target_bir_lowering=False)
    io = {
        "x_t": nc.dram_tensor("x_t", [B, C, T], F16, kind="ExternalInput"),
        "w_qkv_t": nc.dram_tensor("w_qkv_t", [C, 6, 128], F16,
                                  kind="ExternalInput"),
        "w_proj_t": nc.dram_tensor("w_proj_t", [C, C], F16,
                                   kind="ExternalInput"),
        "cc": nc.dram_tensor("cc", [128, T], F16, kind="ExternalInput"),
        "ss": nc.dram_tensor("ss", [128, T], F16, kind="ExternalInput"),
        "tri": nc.dram_tensor("tri", [128, 128], F16, kind="ExternalInput"),
        "y": nc.dram_tensor("y", [B, HL, 128, C], F32, kind="ExternalOutput"),
    }
    with tile.TileContext(nc) as tc, ExitStack() as ctx:
        pools = {
            "const": ctx.enter_context(tc.tile_pool(name="const", bufs=1)),
            "ps_mm": ctx.enter_context(
                tc.tile_pool(name="ps_mm", bufs=2, space="PSUM")),
            "ps_s": ctx.enter_context(
                tc.tile_pool(name="ps_s", bufs=2, space="PSUM")),
            "ps_o": ctx.enter_context(
                tc.tile_pool(name="ps_o", bufs=2, space="PSUM")),
            "ps_d": ctx.enter_context(
                tc.tile_pool(name="ps_d", bufs=2, space="PSUM")),
            "xp": ctx.enter_context(tc.tile_pool(name="xp", bufs=2)),
            "qkvp": ctx.enter_context(tc.tile_pool(name="qkvp", bufs=2)),
            "rp": ctx.enter_context(tc.tile_pool(name="rp", bufs=2)),
            "ep": ctx.enter_context(tc.tile_pool(name="ep", bufs=4)),
            "outp": ctx.enter_context(tc.tile_pool(name="outp", bufs=1)),
            "wp": ctx.enter_context(tc.tile_pool(name="wp", bufs=3)),
            "yp": ctx.enter_context(tc.tile_pool(name="yp", bufs=3)),
            "dnp": ctx.enter_context(tc.tile_pool(name="dnp", bufs=2)),
            "eap": ctx.enter_context(tc.tile_pool(name="eap", bufs=2)),
        }
        _emit(nc, io, pools, mybir)
    nc.compile()
    return nc


def _make_executor(nc):
    import jax
    from jax.sharding import Mesh, PartitionSpec
    from jax.experimental.shard_map import shard_map
    import concourse.mybir as mybir
    from concourse.bass2jax import (
        _bass_exec_p,
        install_neuronx_cc_hook,
        partition_id_tensor,
    )

    install_neuronx_cc_hook()
    partition_name = (
        nc.partition_id_tensor.name if nc.partition_id_tensor else None
    )
    in_names, out_names, out_avals, zero_outs = [], [], [], []
    for alloc in nc.m.functions[0].allocations:
        if not isinstance(alloc, mybir.MemoryLocationSet):
            continue
        name = alloc.memorylocations[0].name
        if alloc.kind == "ExternalInput":
            if name != partition_name:
                in_names.append(name)
        elif alloc.kind == "ExternalOutput":
            shape = tuple(alloc.tensor_shape)
            dtype = mybir.dt.np(alloc.dtype)
            out_names.append(name)
            out_avals.append(jax.core.ShapedArray(shape, dtype))
            zero_outs.append(np.zeros(shape, dtype))
    n_params = len(in_names)
    n_outs = len(out_avals)
    in_names.extend(out_names)
    if partition_name is not None:
        in_names.append(partition_name)
    donate = tuple(range(n_params, n_params + n_outs))

    def _body(*args):
        operands = list(args)
        if partition_name is not None:
            operands.append(partition_id_tensor())
        return tuple(
            _bass_exec_p.bind(
                *operands,
                out_avals=tuple(out_avals),
                in_names=tuple(in_names),
                out_names=tuple(out_names),
                lowering_input_output_aliases=(),
                sim_require_finite=True,
                sim_require_nnan=True,
                nc=nc,
            )
        )

    devices = jax.devices()[:N_CORES]
    assert len(devices) == N_CORES, f"need {N_CORES} cores, got {len(devices)}"
    mesh = Mesh(np.asarray(devices), ("core",))
    in_specs = (PartitionSpec("core"),) * (n_params + n_outs)
    out_specs = (PartitionSpec("core"),) * len(out_names)
    sharded = jax.jit(
        shard_map(_body, mesh=mesh, in_specs=in_specs, out_specs=out_specs,
                  check_rep=False),
        donate_argnums=donate,
        keep_unused=True,
    )

    def run(in_maps):
        per_core = [
            [np.asarray(m[name]) for name in in_names[:n_params]]
            for m in in_maps
        ]
        concat_in = [
            np.concatenate([per_core[c][i] for c in range(N_CORES)], axis=0)
            for i in range(n_params)
        ]
        concat_zeros = [
            np.zeros((N_CORES * z.shape[0], *z.shape[1:]), z.dtype)
            for z in zero_outs
        ]
        out_arrs = sharded(*concat_in, *concat_zeros)
        jax.block_until_ready(out_arrs)
        return [
            {
                name: np.asarray(out_arrs[i]).reshape(
                    N_CORES, *out_avals[i].shape
                )[c]
                for i, name in enumerate(out_names)
            }
            for c in range(N_CORES)
        ]

    return run


def _host_prep(x, w_qkv, w_proj):
    x = np.asarray(x, dtype=np.float32)
    w_qkv = np.asarray(w_qkv, dtype=np.float32)
    w_proj = np.asarray(w_proj, dtype=np.float32)

    x_t = np.ascontiguousarray(x.transpose(0, 2, 1)).astype(np.float16)
    w_proj_t = np.ascontiguousarray(w_proj.T).astype(np.float16)

    pos = np.arange(T, dtype=np.float32)[:, None]
    inv = np.exp(
        np.arange(0, D, 2, dtype=np.float32) * np.float32(-math.log(10000.0) / D)
    )
    ang = pos * inv                                  # [T, 64]
    cosT = np.cos(ang).astype(np.float32).T          # [64, T]
    sinT = np.sin(ang).astype(np.float32).T
    cc = np.concatenate([cosT, cosT], axis=0).astype(np.float16)   # [128, T]
    ss = np.concatenate([-sinT, sinT], axis=0).astype(np.float16)  # [128, T]
    tri = np.triu(np.ones((128, 128), dtype=np.float32)).astype(np.float16)

    in_maps = []
    for c in range(N_CORES):
        h0, h1 = 2 * c, 2 * c + 1
        blocks = []
        for base in (0, C, 2 * C):  # q, k, v feature rows
            for h in (h0, h1):
                blocks.append(w_qkv[base + h * D : base + (h + 1) * D, :])
        w_slab = np.stack(blocks, 0)  # [6, 128, C]
        w_t = np.ascontiguousarray(w_slab.transpose(2, 0, 1)).astype(np.float16)
        in_maps.append(
            {
                "x_t": x_t,
                "w_qkv_t": w_t,
                "w_proj_t": w_proj_t,
                "cc": cc,
                "ss": ss,
                "tri": tri,
            }
        )
    return in_maps


def _gather(outs):
    y = np.empty((B, T, C), dtype=np.float32)
    for c in range(N_CORES):
        for hl in range(HL):
            h = 2 * c + hl
            y[:, h * 128 : (h + 1) * 128, :] = outs[c]["y"][:, hl]
    return y


def kernel(x, w_qkv, w_proj):
    """Full inputs in, full output out. Shards over 8 NeuronCores inside."""
    if "run" not in _CACHE:
        nc = _build()
        _CACHE["run"] = _make_executor(nc)
    run = _CACHE["run"]
    in_maps = _host_prep(x, w_qkv, w_proj)
    outs = run(in_maps)
    return _gather(outs)


# revision 20
# speedup vs baseline: 1.0272x; 1.0272x over previous
"""TRN2 Bass kernel for nn_MultiHeadAttention_86878598464357.

reference:  qkv = x @ w_qkv.T (RoPE on q,k) -> causal softmax attention ->
            torch-faithful reshape [B,H,T,D]->[B,T,C] -> proj @ w_proj.T

Sharding (8 NeuronCores): tensor-parallel over heads, 2 heads per core.
Because the torch-faithful reshape makes output row t' depend only on head
t'//128, each core independently computes full output rows for its heads --
no collectives.

Per core (all fp16 operands, fp32 PSUM accumulation):
  - qkv projection for its 2 heads
  - RoPE: one scalar PSUM->SBUF fp16 copy, then 4 fp16 vector ops against
    host-precomputed duplicated cos/sin tables (2x DVE mode)
  - causal attention in transposed-score layout S^T[s,t]: exp on scalar
    engine (scores O(6), fp32-safe without max subtraction), denominator
    via per-j ones-matmul accumulated in PSUM, reciprocal via [1,TB] ->
    [128,TB//128] DMA round-trip, gpsimd partition-broadcast, normalize TT
  - output projection with stride-16 lhsT access implementing the reshape
Emission order software-pipelines: attention starts inside the qkv(0)
phase, qkv(1) weaves with remaining attn(0), proj weaves with attn(1),
and the denominator chain of block i is emitted between j-units of block
i+1 so no engine queue blocks on the DMA round-trip latency.
"""
import math
from contextlib import ExitStack

import numpy as np

B, T, C = 2, 2048, 2048
H, D = 16, 128
HL = 2
TB = 512
NTB = T // TB
NTT = T // 128
KT = C // 128
SCALE = 1.0 / math.sqrt(D)
N_CORES = 8

_CACHE = {}


def _take(gen, n):
    """Pull and run up to n units from a generator of thunks."""
    for _ in range(n):
        f = next(gen, None)
        if f is None:
            return False
        f()
    return True


def _weave(gen_a, gen_b, na, nb):
    """Round-robin: na units from a, nb units from b, until both dry."""
    alive_a = alive_b = True
    while alive_a or alive_b:
        if alive_a:
            alive_a = _take(gen_a, na)
        if alive_b:
            alive_b = _take(gen_b, nb)


def _emit(nc, io, p, mybir):
    F32 = mybir.dt.float32
    F16 = mybir.dt.float16

    # ---- constants + ACT table prefetch ----
    ones_sb = p["const"].tile([128, 1], F16, name="ones_sb")
    nc.vector.memset(ones_sb[:], 1.0)
    ones_f32 = p["const"].tile([128, 1], F32, name="ones_f32")
    nc.vector.memset(ones_f32[:], 1.0)
    warm = p["const"].tile([128, 1], F32, name="warm")
    # dummy exp: forces the ACT table load at t=0, hidden under initial DMAs
    nc.scalar.activation(warm[:], ones_sb[:], mybir.ActivationFunctionType.Exp)

    w_sb = p["const"].tile([128, KT, 6 * 128], F16, name="w_sb")

    def load_w(fp, q, eng=None):
        ks = slice(q * 4, q * 4 + 4)
        (eng or nc.sync).dma_start(
            out=w_sb[:, ks, fp * 256 : (fp + 1) * 256],
            in_=io["w_qkv_t"][
                ks.start * 128 : ks.stop * 128, fp * 2 : fp * 2 + 2
            ].rearrange("(kt p) f d -> p kt (f d)", p=128),
        )

    cc_sb = p["const"].tile([128, T], F16, name="cc_sb")
    ss_sb = p["const"].tile([128, T], F16, name="ss_sb")
    tri_sb = p["const"].tile([128, 128], F16, name="tri_sb")

    def load_tables():
        nc.gpsimd.dma_start(out=cc_sb[:], in_=io["cc"][:])
        nc.gpsimd.dma_start(out=ss_sb[:], in_=io["ss"][:])
        nc.gpsimd.dma_start(out=tri_sb[:], in_=io["tri"][:])

    def wslice(kt, fb):
        return w_sb[:, kt, fb * 128 : (fb + 1) * 128]

    qkv_t = {}
    out_sb = {}
    last_dn = {}

    # ---------------- qkv projection + rope ----------------
    def qkv_units(b):
        q = {h: p["qkvp"].tile([128, T], F16, name=f"q_sb_{h}") for h in range(HL)}
        k = {h: p["qkvp"].tile([128, T], F16, name=f"k_sb_{h}") for h in range(HL)}
        v = p["qkvp"].tile([128, NTT, HL * 128], F16, name="v_sb")
        qkv_t[b] = (q, k, v)
        x_holder = {}

        def load_x(tb, quarter=None):
            def f():
                if quarter is None or quarter == 0:
                    x_holder[tb] = p["xp"].tile([128, KT, TB], F16, name="x_sb")
                x_sb = x_holder[tb]
                ks = (slice(0, KT) if quarter is None
                      else slice(quarter * 4, quarter * 4 + 4))
                nc.sync.dma_start(
                    out=x_sb[:, ks],
                    in_=io["x_t"][b][
                        ks.start * 128 : ks.stop * 128, tb * TB : (tb + 1) * TB
                    ].rearrange("(kt p) t -> p kt t", p=128),
                )
            return f

        psum_hold = {}

        def qk_mms(tb, fb, k0, k1):
            def f():
                x_sb = x_holder[tb]
                if k0 == 0:
                    psum_hold[fb] = p["ps_mm"].tile([128, TB], F32,
                                                    name="qk_psum", tag="mmps")
                psum = psum_hold[fb]
                for kt in range(k0, k1):
                    nc.tensor.matmul(
                        psum[:],
                        wslice(kt, fb),
                        rhs=x_sb[:, kt],
                        start=(kt == 0),
                        stop=(kt == KT - 1),
                    )
            return f

        def qk_rope(tb, fb, dst, h):
            def f():
                ts = slice(tb * TB, (tb + 1) * TB)
                psum = psum_hold.pop(fb)
                # rope: dst = psum*cc + swap(psum)*ss; swap via scalar copies
                # (PSUM operands are exempt from the equal-base-partition rule)
                qsw = p["rp"].tile([128, TB], F16, name="qsw")
                nc.scalar.copy(qsw[0:64, :], psum[64:128, :])
                nc.scalar.copy(qsw[64:128, :], psum[0:64, :])
                t1 = p["rp"].tile([128, TB], F16, name="rope_t1")
                t2 = p["rp"].tile([128, TB], F16, name="rope_t2")
                nc.vector.tensor_mul(t1[:], psum[:], cc_sb[:, ts])
                nc.vector.tensor_mul(t2[:], qsw[:], ss_sb[:, ts])
                nc.vector.tensor_add(dst[h][:, ts], t1[:], t2[:])
            return f

        def qk_chunk(tb, fb, dst, h):
            mm = qk_mms(tb, fb, 0, KT)
            rp = qk_rope(tb, fb, dst, h)
            def f():
                mm()
                rp()
            return f

        def v_chunk(tb, tl):
            def f():
                x_sb = x_holder[tb]
                tt = tb * 4 + tl
                psum = p["ps_mm"].tile([128, HL * 128], F32, name="v_psum",
                                       tag="mmps")
                for kt in range(KT):
                    nc.tensor.matmul(
                        psum[:],
                        x_sb[:, kt, tl * 128 : (tl + 1) * 128],
                        rhs=w_sb[:, kt, 4 * 128 : 6 * 128],
                        start=(kt == 0),
                        stop=(kt == KT - 1),
                    )
                nc.scalar.copy(v[:, tt], psum[:])
            return f

        fbs = [(q, 0), (q, 1), (k, 0), (k, 1)]
        for tb in range(NTB):
            if b == 0 and tb == 0:
                # cold start: minimal-data half-chains so PE starts after
                # only x[kt0-7] (1MB) + w[q-pair,kt0-7] (512KB) have landed
                yield load_x(tb, quarter=0)
                yield load_x(tb, quarter=1)
                yield lambda: load_w(0, 0)
                yield lambda: load_w(0, 1)
                yield load_x(tb, quarter=2)
                yield load_x(tb, quarter=3)
                yield lambda: load_w(0, 2, nc.gpsimd)
                yield lambda: load_w(0, 3, nc.gpsimd)
                yield lambda: load_w(1, 0)
                yield lambda: load_w(1, 1)
                yield lambda: load_w(1, 2)
                yield lambda: load_w(1, 3)
                yield load_tables
                for q_ in range(4):
                    yield lambda q_=q_: load_w(2, q_)
                for pair in (0, 2):
                    a, b_ = pair, pair + 1
                    yield qk_mms(tb, a, 0, 8)
                    yield qk_mms(tb, b_, 0, 8)
                    yield qk_mms(tb, a, 8, KT)
                    yield qk_rope(tb, a, fbs[a][0], fbs[a][1])
                    yield qk_mms(tb, b_, 8, KT)
                    yield qk_rope(tb, b_, fbs[b_][0], fbs[b_][1])
            else:
                yield load_x(tb)
                for fb, (dst, h) in enumerate(fbs):
                    yield qk_chunk(tb, fb, dst, h)
            for tl in range(4):
                yield v_chunk(tb, tl)

    # ---------------- attention ----------------
    def alloc_out(b):
        for h in range(HL):
            out_sb[(b, h)] = p["outp"].tile([128, T], F16, name=f"o_sb_{b}_{h}")

    def attn_units(b, blocks):
        """Yield j-units for the given (h, tb) blocks; denominator epilogues
        of block i are yielded interleaved between units of block i+1."""
        pending = []  # deferred epilogue thunks

        for h, tb in blocks:
            q, k, v = qkv_t[b]
            o_sb = out_sb[(b, h)]
            ts = slice(tb * TB, (tb + 1) * TB)
            o_ps = p["ps_o"].tile([128, TB], F32, name="o_ps", tag="ops")
            ea = {}
            njs = tb * 4 + 4

            for j in range(njs):
                def f(h=h, tb=tb, j=j, o_ps=o_ps, ea=ea, njs=njs):
                    c0 = max(0, j * 128 - tb * TB)
                    cs = slice(c0, TB)
                    tcs = slice(tb * TB + c0, (tb + 1) * TB)
                    s_ps = p["ps_s"].tile([128, TB], F32, name="s_ps", tag="sps")
                    nc.tensor.matmul(
                        s_ps[:, cs],
                        k[h][:, j * 128 : (j + 1) * 128],
                        rhs=q[h][:, tcs],
                        start=True,
                        stop=True,
                    )
                    e_sb = p["ep"].tile([128, TB], F16, name="e_sb", tag="e")
                    nc.scalar.activation(
                        e_sb[:, cs],
                        s_ps[:, cs],
                        mybir.ActivationFunctionType.Exp,
                        scale=SCALE,
                    )
                    if j >= tb * 4:
                        dcs = slice(c0, c0 + 128)
                        nc.vector.tensor_mul(e_sb[:, dcs], e_sb[:, dcs], tri_sb[:])
                    nc.tensor.matmul(
                        o_ps[:, cs],
                        v[:, j, h * 128 : (h + 1) * 128],
                        rhs=e_sb[:, cs],
                        start=(j == 0),
                        stop=(j == njs - 1),
                    )
                    # denominator partials accumulate on the vector engine
                    if j == 0:
                        ea["t"] = p["eap"].tile([128, TB], F16, name="eacc")
                        nc.vector.tensor_copy(ea["t"][:], e_sb[:])
                    else:
                        nc.vector.tensor_add(
                            ea["t"][:, cs], ea["t"][:, cs], e_sb[:, cs]
                        )
                yield f
                if pending and j % 2 == 1:
                    yield pending.pop(0)

            # epilogue for this block, deferred into the next block's units
            dn = {}

            def epi1(ea=ea, dn=dn):
                d_ps = p["ps_d"].tile([1, TB], F32, name="d_ps", tag="dps")
                nc.tensor.matmul(d_ps[:], ones_sb[:], rhs=ea["t"][:],
                                 start=True, stop=True)
                dn["den_sb"] = p["dnp"].tile([1, TB], F32, name="den_sb")
                nc.vector.tensor_copy(dn["den_sb"][:], d_ps[:])
                last_dn["dn"] = dn
                dn["den128"] = p["dnp"].tile([128, TB // 128], F32, name="den128")
                nc.sync.dma_start(out=dn["den128"][:], in_=dn["den_sb"][0:1, :])

            def epi2(dn=dn):
                dn["rec128"] = p["dnp"].tile([128, TB // 128], F32, name="rec128")
                nc.vector.reciprocal(dn["rec128"][:], dn["den128"][:])
                dn["rec_sb"] = p["dnp"].tile([1, TB], F32, name="rec_sb")
                nc.sync.dma_start(out=dn["rec_sb"][0:1, :], in_=dn["rec128"][:])

            def epi3(dn=dn, o_ps=o_ps, o_sb=o_sb, ts=ts):
                rb = p["dnp"].tile([128, TB], F32, name="rb_sb")
                nc.gpsimd.partition_broadcast(rb[:], dn["rec_sb"][:])
                nc.vector.tensor_mul(o_sb[:, ts], o_ps[:], rb[:])

            pending.extend([epi1, epi2, epi3])

        for f in pending:
            yield f

    # ---------------- output projection ----------------
    wt_tiles = {}
    wt_order = []

    def proj_units(jobs, gate_first=False, split_loads=False):
        """jobs: list of (b, ob, h); loads wt when ob not resident (bufs=3).
        gate_first: prefix the first chunk with a 1-col matmul reading the
        last attention block's denominator -- FIFO-forces these chunks into
        the kernel-tail window instead of being hoisted as gap fillers."""
        gate = [gate_first]

        def load_wt(ob):
            def f():
                wt = p["wp"].tile([128, KT, TB], F16, name="wt")
                if split_loads:
                    for half, eng in ((0, nc.sync), (1, nc.gpsimd)):
                        ks = slice(half * 8, half * 8 + 8)
                        eng.dma_start(
                            out=wt[:, ks],
                            in_=io["w_proj_t"][
                                ks.start * 128 : ks.stop * 128,
                                ob * TB : (ob + 1) * TB,
                            ].rearrange("(kt p) o -> p kt o", p=128),
                        )
                else:
                    nc.sync.dma_start(
                        out=wt[:],
                        in_=io["w_proj_t"][:, ob * TB : (ob + 1) * TB].rearrange(
                            "(kt p) o -> p kt o", p=128
                        ),
                    )
                wt_tiles[ob] = wt
            return f

        def pchunk(b, ob, h):
            def f():
                wt = wt_tiles[ob]
                os_ = slice(ob * TB, (ob + 1) * TB)
                y_ps = p["ps_mm"].tile([128, TB], F32, name="y_ps", tag="mmps")
                if gate[0]:
                    gate[0] = False
                    nc.tensor.matmul(
                        y_ps[0:1, 0:1], ones_f32[0:1, 0:1],
                        rhs=last_dn["dn"]["den_sb"][0:1, 0:1],
                        start=True, stop=True,
                    )
                o_sb = out_sb[(b, h)]
                for kt in range(KT):
                    lhsT = o_sb.rearrange("d (t2 g) -> d g t2", g=16)[:, kt]
                    nc.tensor.matmul(
                        y_ps[:],
                        lhsT,
                        rhs=wt[:, kt],
                        start=(kt == 0),
                        stop=(kt == KT - 1),
                    )
                y_sb = p["yp"].tile([128, TB], F32, name="y_sb")
                nc.vector.tensor_copy(y_sb[:], y_ps[:])
                nc.gpsimd.dma_start(out=io["y"][b, h, :, os_], in_=y_sb[:])
            return f

        for job in jobs:
            if job[0] == "load":          # explicit prefetch entry
                yield load_wt(job[1])
                wt_order.append(job[1])
                continue
            b, ob, h = job
            if ob not in wt_order[-3:]:   # mirrors wp pool bufs=3 residency
                yield load_wt(ob)
                wt_order.append(ob)
            yield pchunk(b, ob, h)

    # ---------------- schedule ----------------
    NOB = C // TB
    g_qkv0 = qkv_units(0)
    _take(g_qkv0, 35)                       # tb=0 fully
    alloc_out(0)
    g_attn0 = attn_units(0, [(h, tb) for tb in range(NTB) for h in range(HL)])
    _weave(g_qkv0, g_attn0, 3, 3)           # qkv(0) tb1-3 with early attn(0)
    g_qkv1 = qkv_units(1)
    _weave(g_qkv1, g_attn0, 1, 2)           # qkv(1) with remaining attn(0)

    alloc_out(1)
    # attn(1): h0's smallest block goes LAST so that by the time it ends,
    # h1 is complete and ~20us of proj(1,h1)+proj(0,ob3) chunks sit behind
    # it in the PE FIFO, covering the trailing denominator chains.
    g_attn1 = attn_units(
        1, [(0, 3), (0, 2), (0, 1)] + [(1, tb) for tb in (3, 2, 1, 0)]
        + [(0, 0)]
    )
    g_proj0 = proj_units([(0, ob, h) for ob in range(NOB) for h in range(HL)])
    _weave(g_attn1, g_proj0, 8, 1)          # attn(1) with all of proj(0)

    # mid chunks (h1-gated): ready as soon as h1 closes; they sit behind
    # the trailing den chains in the PE FIFO and cover their latency
    _take(proj_units([(1, 3, 1), (1, 1, 1), (1, 2, 1), (1, 0, 1)]), 100)
    # tail: proj(1,h0); ring-aware order, reload prefetched 2 chunks early
    _take(proj_units([(1, 2, 0), ("load", 1), (1, 0, 0), (1, 3, 0),
                      (1, 1, 0)], split_loads=True), 100)


def _build():
    from concourse import bacc
    import concourse.mybir as mybir
    import concourse.tile as tile

    F32 = mybir.dt.float32
    F16 = mybir.dt.float16

    nc = bacc.Bacc(None, target_bir_lowering=False)
    io = {
        "x_t": nc.dram_tensor("x_t", [B, C, T], F16, kind="ExternalInput"),
        "w_qkv_t": nc.dram_tensor("w_qkv_t", [C, 6, 128], F16,
                                  kind="ExternalInput"),
        "w_proj_t": nc.dram_tensor("w_proj_t", [C, C], F16,
                                   kind="ExternalInput"),
        "cc": nc.dram_tensor("cc", [128, T], F16, kind="ExternalInput"),
        "ss": nc.dram_tensor("ss", [128, T], F16, kind="ExternalInput"),
        "tri": nc.dram_tensor("tri", [128, 128], F16, kind="ExternalInput"),
        "y": nc.dram_tensor("y", [B, HL, 128, C], F32, kind="ExternalOutput"),
    }
    with tile.TileContext(nc) as tc, ExitStack() as ctx:
        pools = {
            "const": ctx.enter_context(tc.tile_pool(name="const", bufs=1)),
            "ps_mm": ctx.enter_context(
                tc.tile_pool(name="ps_mm", bufs=2, space="PSUM")),
            "ps_s": ctx.enter_context(
                tc.tile_pool(name="ps_s", bufs=2, space="PSUM")),
            "ps_o": ctx.enter_context(
                tc.tile_pool(name="ps_o", bufs=2, space="PSUM")),
            "ps_d": ctx.enter_context(
                tc.tile_pool(name="ps_d", bufs=2, space="PSUM")),
            "xp": ctx.enter_context(tc.tile_pool(name="xp", bufs=2)),
            "qkvp": ctx.enter_context(tc.tile_pool(name="qkvp", bufs=2)),
            "rp": ctx.enter_context(tc.tile_pool(name="rp", bufs=2)),
            "ep": ctx.enter_context(tc.tile_pool(name="ep", bufs=4)),
            "outp": ctx.enter_context(tc.tile_pool(name="outp", bufs=1)),
            "wp": ctx.enter_context(tc.tile_pool(name="wp", bufs=3)),
            "yp": ctx.enter_context(tc.tile_pool(name="yp", bufs=3)),
            "dnp": ctx.enter_context(tc.tile_pool(name="dnp", bufs=2)),
            "eap": ctx.enter_context(tc.tile_pool(name="eap", bufs=2)),
        }
        _emit(nc, io, pools, mybir)
    nc.compile()
    return nc


def _make_executor(nc):
    import jax
    from jax.sharding import Mesh, PartitionSpec
    from jax.experimental.shard_map import shard_map
    import concourse.mybir as mybir
    from concourse.bass2jax import (
        _bass_exec_p,
        install_neuronx_cc_hook,
        partition_id_tensor,
    )

    install_neuronx_cc_hook()
    partition_name = (
        nc.partition_id_tensor.name if nc.partition_id_tensor else None
    )
    in_names, out_names, out_avals, zero_outs = [], [], [], []
    for alloc in nc.m.functions[0].allocations:
        if not isinstance(alloc, mybir.MemoryLocationSet):
            continue
        name = alloc.memorylocations[0].name
        if alloc.kind == "ExternalInput":
            if name != partition_name:
                in_names.append(name)
        elif alloc.kind == "ExternalOutput":
            shape = tuple(alloc.tensor_shape)
            dtype = mybir.dt.np(alloc.dtype)
            out_names.append(name)
            out_avals.append(jax.core.ShapedArray(shape, dtype))
            zero_outs.append(np.zeros(shape, dtype))
    n_params = len(in_names)
    n_outs = len(out_avals)
    in_names.extend(out_names)
    if partition_name is not None:
        in_names.append(partition_name)
    donate = tuple(range(n_params, n_params + n_outs))

    def _body(*args):
        operands = list(args)
        if partition_name is not None:
            operands.append(partition_id_tensor())
        return tuple(
            _bass_exec_p.bind(
                *operands,
                out_avals=tuple(out_avals),
                in_names=tuple(in_names),
                out_names=tuple(out_names),
                lowering_input_output_aliases=(),
                sim_require_finite=True,
                sim_require_nnan=True,
                nc=nc,
            )
        )

    devices = jax.devices()[:N_CORES]
    assert len(devices) == N_CORES, f"need {N_CORES} cores, got {len(devices)}"
    mesh = Mesh(np.asarray(devices), ("core",))
    in_specs = (PartitionSpec("core"),) * (n_params + n_outs)
    out_specs = (PartitionSpec("core"),) * len(out_names)
    sharded = jax.jit(
        shard_map(_body, mesh=mesh, in_specs=in_specs, out_specs=out_specs,
                  check_rep=False),
        donate_argnums=donate,
        keep_unused=True,
    )

    def run(in_maps):
        per_core = [
            [np.asarray(m[name]) for name in in_names[:n_params]]
            for m in in_maps
        ]
        concat_in = [
            np.concatenate([per_core[c][i] for c in range(N_CORES)], axis=0)
            for i in range(n_params)
        ]
        concat_zeros = [
            np.zeros((N_CORES * z.shape[0], *z.shape[1:]), z.dtype)
            for z in zero_outs
        ]
        out_arrs = sharded(*concat_in, *concat_zeros)
        jax.block_until_ready(out_arrs)
        return [
            {
                name: np.asarray(out_arrs[i]).reshape(
                    N_CORES, *out_avals[i].shape
                )[c]
                for i, name in enumerate(out_names)
            }
            for c in range(N_CORES)
        ]

    return run


def _host_prep(x, w_qkv, w_proj):
    x = np.asarray(x, dtype=np.float32)
    w_qkv = np.asarray(w_qkv, dtype=np.float32)
    w_proj = np.asarray(w_proj, dtype=np.float32)

    x_t = np.ascontiguousarray(x.transpose(0, 2, 1)).astype(np.float16)
    w_proj_t = np.ascontiguousarray(w_proj.T).astype(np.float16)

    pos = np.arange(T, dtype=np.float32)[:, None]
    inv = np.exp(
        np.arange(0, D, 2, dtype=np.float32) * np.float32(-math.log(10000.0) / D)
    )
    ang = pos * inv                                  # [T, 64]
    cosT = np.cos(ang).astype(np.float32).T          # [64, T]
    sinT = np.sin(ang).astype(np.float32).T
    cc = np.concatenate([cosT, cosT], axis=0).astype(np.float16)   # [128, T]
    ss = np.concatenate([-sinT, sinT], axis=0).astype(np.float16)  # [128, T]
    tri = np.triu(np.ones((128, 128), dtype=np.float32)).astype(np.float16)

    in_maps = []
    for c in range(N_CORES):
        h0, h1 = 2 * c, 2 * c + 1
        blocks = []
        for base in (0, C, 2 * C):  # q, k, v feature rows
            for h in (h0, h1):
                blocks.append(w_qkv[base + h * D : base + (h + 1) * D, :])
        w_slab = np.stack(blocks, 0)  # [6, 128, C]
        w_t = np.ascontiguousarray(w_slab.transpose(2, 0, 1)).astype(np.float16)
        in_maps.append(
            {
                "x_t": x_t,
                "w_qkv_t": w_t,
                "w_proj_t": w_proj_t,
                "cc": cc,
                "ss": ss,
                "tri": tri,
            }
        )
    return in_maps


def _gather(outs):
    y = np.empty((B, T, C), dtype=np.float32)
    for c in range(N_CORES):
        for hl in range(HL):
            h = 2 * c + hl
            y[:, h * 128 : (h + 1) * 128, :] = outs[c]["y"][:, hl]
    return y


def kernel(x, w_qkv, w_proj):
    """Full inputs in, full output out. Shards over 8 NeuronCores inside."""
    if "run" not in _CACHE:
        nc = _build()
        _CACHE["run"] = _make_executor(nc)
    run = _CACHE["run"]
    in_maps = _host_prep(x, w_qkv, w_proj)
    outs = run(in_maps)
    return _gather(outs)
